# revision 140
# baseline (speedup 1.0000x reference)
"""Causal multi-head attention block (GPT-style) on 8 TRN2 NeuronCores.

Sharding: core (b, g) = batch b in {0,1} x head-group g in {0..3} (4 heads of
dh=64 each). Megatron-style: each core computes q/k/v projections for its 256
channels, attention for its 4 heads, and a partial c_proj using its 256 rows of
W_proj. Host sums the 4 partial projections per batch (+ bias terms).

On-core dataflow.  Precision plan: all q/k projections run fp8e4m3
DoubleRow (host-prefolded 256-channel pairs, W pre-scaled x32 out of the
subnormal range, 0.5 PE cycles/row); slices 1-3 also SCORE in fp8
DoubleRow — their >=512-key softmaxes average the ~3% noise away — while
slice 0's near-diagonal scores stay bf16.  The V path and x's v-copy stay
bf16 end to end (v errors hit the output absmax directly); output is bf16.
  qT,kT = (W8 stationary) @ x8          -> psum -> bf16 (slice 0) and fp8
                                           key-pair tiles [32p, 2, .] built
                                           with 8 small SBUF->SBUF DMAs
  v     = (xT stationary) @ W_v         -> [2048, 4*65] bf16 (ones col appended)
  sT    = kT_tile.T @ qT_slice          -> scores transposed [j, i],
                                           exact causal column range
  u     = exp(sT) -> bf16 (no max-subtraction: scores are O(3); diagonal
                    boundary masked by a [128,128] triangle multiply)
  av    = (u_tile stationary [k, q128]) @ (v|1 moving [k, 65])
          -> psum [128 q, 4 heads x 65]: cols 0-63 unnormalized attn-out,
          col 64 = softmax sum.  Query-stationary orientation: 65-col moving
          operands cost 65 PE cycles each vs 512 for the [65, q] orientation.
  a     = av[:, h*65:h*65+64] * (1/av[:, h*65+64]) per-partition scalar (DVE)
  aT    = PE transpose(a)               -> [128 d, 128 q] per (qt, head-pair)
  out   = (aT stationary) @ W_proj_rows -> partial [2048, 1024] -> bf16 out

Scheduling: the ACT exp stream is the scarce serial resource.  Scheduler
priorities (lower wins among READY instructions) keep it fed: each slice's
q/k projection chain (matmuls, psum->sbuf copies, fp8 rearrange DMAs) is
placed ahead of or inside the score band so a slice boundary never starves
ACT; v/av/norm run at middle priority; all c_proj work floats at the back
as PE filler for the ACT-heavy late windows.  Engine placement respects
hardware limits (GPSIMD cannot touch PSUM): psum reads go to DVE/ACT, with
the exp-idle windows (early slices, post-exp tail) absorbing copies on ACT.
DMAs are packed into few fat transfers (dispatch is ~650ns serialized per
dma_start regardless of size), ordered x(slice0)+Wqk -> x(slice1) -> Wv ->
aux -> Wp by first-use time.
"""

import sys

try:
    import concourse  # noqa: F401
except ImportError:
    sys.path.insert(0, "/opt/trn_rl_repo")

from contextlib import ExitStack

import numpy as np

import concourse.tile as tile
from concourse import bacc, mybir
from concourse.bass_utils import run_bass_kernel_spmd

import ml_dtypes

F32 = mybir.dt.float32
F32R = mybir.dt.float32r
BF16 = mybir.dt.bfloat16
FP8 = mybir.dt.float8e4
EXP = mybir.ActivationFunctionType.Exp
MUL = mybir.AluOpType.mult
ADD = mybir.AluOpType.add
DROW = mybir.MatmulPerfMode.DoubleRow

B, T, D = 2, 2048, 1024
HG, DH = 4, 64          # heads per core, head dim
CQK = 512               # q+k channels per core
CV = 256                # v channels per core
KT = D // 128           # contraction tiles of the projections
TS = 512                # t-slice width
NTS = T // TS
NT128 = T // 128
VW = HG * (DH + 1)      # 260: per-t-tile v row (4 heads x (64 v + 1 ones))


def interleave(primary, filler, back=0.45):
    """Merge filler among primary units, weighted toward the back where the
    ACT pipeline debt is largest."""
    if not filler:
        return list(primary)
    n = len(primary)
    out = []
    fi = 0
    for i, p in enumerate(primary):
        out.append(p)
        x = (i + 1) / n
        want = ((1 - back) * x + back * x * x) * len(filler)
        while fi < len(filler) and fi + 1 <= want:
            out.append(filler[fi])
            fi += 1
    out.extend(filler[fi:])
    return out


def build():
    nc = bacc.Bacc(None)

    # DMA dispatch is ~650ns of serialized sequencer time per dma_start
    # regardless of size, so inputs are packed for few, fat transfers
    xT_in = nc.dram_tensor("xT", [NTS, 128, KT * TS], BF16, kind="ExternalInput")
    wqk_in = nc.dram_tensor("wqk", [KT, 128, CQK], BF16, kind="ExternalInput")
    wv_in = nc.dram_tensor("wv", [128, KT * CV], BF16, kind="ExternalInput")
    wp_in = nc.dram_tensor("wp", [128, 2 * D], F32R, kind="ExternalInput")
    x8_in = nc.dram_tensor("x8", [NTS, 128, KT * TS], FP8,
                           kind="ExternalInput")
    wqk8_in = nc.dram_tensor("wqk8", [KT // 2, 128, 2 * CQK], FP8,
                             kind="ExternalInput")
    bias_in = nc.dram_tensor("bqk", [128, 4], F32, kind="ExternalInput")
    aux_in = nc.dram_tensor("aux", [128, 384], F32, kind="ExternalInput")
    out_dram = nc.dram_tensor("out", [NT128, 128, 2 * TS], BF16,
                              kind="ExternalOutput")

    with ExitStack() as ctx:
        tc = ctx.enter_context(tile.TileContext(nc))

        const = ctx.enter_context(tc.tile_pool(name="const", bufs=1))
        big = ctx.enter_context(tc.tile_pool(name="big", bufs=1))
        upool = ctx.enter_context(tc.tile_pool(name="upool", bufs=44))
        apool = ctx.enter_context(tc.tile_pool(name="apool", bufs=6))
        atp = ctx.enter_context(tc.tile_pool(name="atp", bufs=33))
        rpool = ctx.enter_context(tc.tile_pool(name="rpool", bufs=4))
        outp = ctx.enter_context(tc.tile_pool(name="outp", bufs=8))
        xrp = ctx.enter_context(tc.tile_pool(name="xrp", bufs=2))
        wqkp = ctx.enter_context(tc.tile_pool(name="wqkp", bufs=1))
        x8p = ctx.enter_context(tc.tile_pool(name="x8p", bufs=2))
        wvp = ctx.enter_context(tc.tile_pool(name="wvp", bufs=1))

        ps_s = ctx.enter_context(tc.tile_pool(name="ps_s", bufs=2, space="PSUM"))
        ps_av = ctx.enter_context(tc.tile_pool(name="ps_av", bufs=2, space="PSUM"))
        ps_mm = ctx.enter_context(tc.tile_pool(name="ps_mm", bufs=2, space="PSUM"))

        # constants
        bias_sb = const.tile([128, 4], F32, tag="bias")
        auxf = const.tile([128, 384], F32, tag="auxf")  # [tri|tri|ident]
        tri = const.tile([128, 256], BF16, tag="tri")
        ident = auxf[:, 256:384]
        ones128 = const.tile([128, 16], BF16, tag="ones128")
        nc.vector.memset(ones128[:], 1.0)
        # preload the ACT exp table off the critical path
        scratch = const.tile([128, 16], BF16, tag="scratch")
        nc.scalar.activation(scratch[:], ones128[:], EXP)

        # persistent intermediates.  bf16 q/k only for slice 0 (its
        # near-diagonal softmaxes average over few keys, so quantization
        # error shows); slices 1-3 score in fp8 DoubleRow, where the ~3%
        # weight error washes out over >=512-key softmaxes.
        qkT = {(ct, 0): big.tile([128, TS], BF16, tag=f"qkT{ct}_0",
                                 name=f"qkT{ct}_0")
               for ct in range(4)}
        # fp8 key-pair layout, one tile per slice: partition half*32+p
        # holds channels half*64+p (s=0 half of the row) and
        # half*64+32+p (s=1); within a half the ct blocks are side by side
        # (slice 0 stores only its k blocks: cts 2,3)
        q8T = {}
        qk8s = ctx.enter_context(tc.tile_pool(name="qk8s", bufs=3))
        q8stage = {}
        Vt = [big.tile([128, 4 * VW], BF16, tag=f"Vt{ts}", name=f"Vt{ts}")
              for ts in range(NTS)]

        # weight + first-slice DMAs.  Slice 0 streams in 8 k-tile chunks so
        # accumulation can start as soon as the first pairs land; everything
        # else is one fat transfer per tensor (dispatch is the scarce
        # resource, not bandwidth).
        xrt, xr8 = {}, {}
        wqk8 = wqkp.tile([128, KT * CQK], FP8, tag="wqk8", name="wqk8")
        xr8[0] = x8p.tile([128, KT * TS], FP8, tag="x8", name="x8_0")
        for j in range(KT // 2):
            nc.sync.dma_start(
                out=xr8[0][:, j * 2 * TS:(j + 1) * 2 * TS],
                in_=x8_in[0][:, j * 2 * TS:(j + 1) * 2 * TS])
            nc.sync.dma_start(
                out=wqk8[:, j * 2 * CQK:(j + 1) * 2 * CQK],
                in_=wqk8_in[j])
        xrt[0] = xrp.tile([128, KT * TS], BF16, tag="xr", name="xr_0")
        # v-path + cold weights ride the SECOND hardware DGE queue (ACT),
        # held until the critical fp8 qk stream has cleared the DMA engine;
        # their dispatches finish on ACT.SEQ before the first exp fires
        with tc.tile_wait_until(0.001):
            for k in range(0, KT, 4):
                nc.scalar.dma_start(out=xrt[0][:, k * TS:(k + 4) * TS],
                                    in_=xT_in[0][:, k * TS:(k + 4) * TS])
        # xr slice 1 right behind slice 0: the slice-1 q-projection gates
        # that slice's whole exp stream, while wv/aux/wp aren't read
        # until later
        nc.sync.dma_start(out=bias_sb[:], in_=bias_in[:])
        xr8[1] = x8p.tile([128, KT * TS], FP8, tag="x8", name="x8_1")
        nc.sync.dma_start(out=xr8[1][:], in_=x8_in[1])
        nc.sync.dma_start(out=auxf[:], in_=aux_in[:])
        xrt[1] = xrp.tile([128, KT * TS], BF16, tag="xr", name="xr_1")
        wvt = wvp.tile([128, KT * CV], BF16, tag="wv", name="wvt")
        wpt = big.tile([128, 2 * D], F32R, tag="wpt", name="wpt")
        with tc.tile_wait_until(0.001):
            for k in range(0, KT, 4):
                nc.scalar.dma_start(out=xrt[1][:, k * TS:(k + 4) * TS],
                                    in_=xT_in[1][:, k * TS:(k + 4) * TS])
            nc.scalar.dma_start(out=wvt[:], in_=wv_in[:])
            nc.scalar.dma_start(out=wpt[:], in_=wp_in[:])
        nc.gpsimd.tensor_copy(tri[:], auxf[:, 0:256])

        def load_xr(ts):
            def unit():
                x1 = xrp.tile([128, KT * TS], BF16, tag="xr", name=f"xr_{ts}")
                for k in range(0, KT, 4):
                    nc.sync.dma_start(out=x1[:, k * TS:(k + 4) * TS],
                                      in_=xT_in[ts][:, k * TS:(k + 4) * TS])
                xrt[ts] = x1
                x2 = x8p.tile([128, KT * TS], FP8, tag="x8", name=f"x8_{ts}")
                nc.sync.dma_start(out=x2[:], in_=x8_in[ts])
                xr8[ts] = x2
            return unit

        q8n = {}

        def emit_qk_copies(ts, ct, ps):
            # The psum->sbuf copies and rearrange DMAs sit on the critical
            # chain to the next slice's exp stream AND gate the ps_mm slot
            # rotation; never let them lose the DVE/queue priority heaps to
            # bulk sc-band work.
            saved_prio = tc.cur_priority
            tc.cur_priority = BAND["sc"] - 10_000 + 10 * (4 * ts + ct)
            """psum -> bf16 (slice 0 only) and/or the fp8 stage tile.  The
            key-pair rearrange DMAs fire per head-pair as soon as that
            pair's q and k blocks are staged (ct order is 0,2,1,3), so the
            hp=0 score stream never waits on the hp=1 projections."""
            scale = 0.125 if ct < 2 else 1.0
            scale /= 32.0   # wqk8 is pre-scaled x32 into fp8 normal range
            if ts == 0:
                nc.vector.tensor_scalar(
                    qkT[ct, 0][:], ps[:],
                    scale, bias_sb[:, ct:ct + 1], op0=MUL, op1=ADD)
            if ct >= 2 or ts >= 1:
                cdim = 2 if ts == 0 else 4
                cx = ct - 2 if ts == 0 else ct
                if ts not in q8stage:
                    q8stage[ts] = qk8s.tile([128, cdim * TS], FP8,
                                            tag="qk8s", name=f"qk8s_{ts}")
                    q8T[ts] = big.tile([128, 2 * cdim * TS], FP8,
                                       tag=f"q8T{ts}", name=f"q8T{ts}")
                    q8n[ts] = 0
                st = q8stage[ts]
                nc.vector.tensor_scalar(
                    st[:, cx * TS:(cx + 1) * TS], ps[:],
                    scale, bias_sb[:, ct:ct + 1], op0=MUL, op1=ADD)
                q8n[ts] += 1
                if ts == 0:
                    # k-only tile [64, (s, h, n)]; after each ct the matching
                    # hp's 4 DMAs can go
                    hp = ct - 2
                elif q8n[ts] == 2:
                    hp = 0   # ct order is 0,2,1,3: q+k of pair 0 staged
                elif q8n[ts] == 4:
                    hp = 1
                else:
                    tc.cur_priority = saved_prio
                    return
                if ts == 0:
                    o4 = q8T[0][:].rearrange("p (s h n) -> p s h n",
                                             s=2, h=2)
                    for half in range(2):
                        for s in range(2):
                            nc.sync.dma_start(
                                out=o4[half * 32:(half + 1) * 32, s, hp],
                                in_=st[half * 64 + s * 32:
                                       half * 64 + s * 32 + 32,
                                       hp * TS:(hp + 1) * TS])
                else:
                    # [64, (s, h, w, n)]: w = q|k for that head pair
                    o5 = q8T[ts][:].rearrange("p (s h w n) -> p s h w n",
                                              s=2, h=2, w=2)
                    i4 = st[:].rearrange("p (c n) -> p c n", c=4)
                    for half in range(2):
                        for s in range(2):
                            nc.sync.dma_start(
                                out=o5[half * 32:(half + 1) * 32, s, hp],
                                in_=i4[half * 64 + s * 32:
                                       half * 64 + s * 32 + 32, hp::2])
            tc.cur_priority = saved_prio

        def v_group(ts, sub):
            def unit():
                ps = ps_mm.tile([128, CV], F32, tag="mm", name=f"v_{ts}_{sub}")
                for k in range(KT):
                    nc.tensor.matmul(
                        ps[:],
                        xrt[ts][:, k * TS + sub * 128:k * TS + sub * 128 + 128],
                        wvt[:, k * CV:(k + 1) * CV],
                        start=(k == 0), stop=(k == KT - 1))
                v3 = Vt[ts][:].rearrange("p (s h e) -> p s h e", h=HG, e=DH + 1)
                nc.vector.tensor_copy(
                    v3[:, sub, :, 0:DH],
                    ps[:].rearrange("p (h e) -> p h e", e=DH))
                if sub == 0:
                    nc.vector.tensor_copy(
                        v3[:, :, :, DH],
                        ones128[:].rearrange("p (s h) -> p s h", h=HG))
            return unit

        utiles = {}

        def att_sc(gi, hp, jt):
            def unit():
                d = jt * 128 - gi * TS
                c0 = max(d, 0)
                ss = ps_s.tile([128, 2 * TS], F32, tag="ss",
                               name=f"ss_{gi}_{hp}_{jt}")
                jts = jt // 4
                jo = (jt % 4) * 128
                for half in range(2):
                    if gi == 0:
                        p0 = half * 64
                        nc.tensor.matmul(
                            ss[:, half * TS + c0:(half + 1) * TS],
                            qkT[2 + hp, jts][p0:p0 + 64, jo:jo + 128],
                            qkT[hp, gi][p0:p0 + 64, c0:TS],
                            start=True, stop=True)
                    else:
                        p0 = half * 32
                        if jts == 0:
                            k8 = q8T[0][p0:p0 + 32, :].rearrange(
                                "p (s h n) -> p s h n", s=2, h=2)[
                                :, :, hp, jo:jo + 128]
                        else:
                            k8 = q8T[jts][p0:p0 + 32, :].rearrange(
                                "p (s h w n) -> p s h w n", s=2, h=2, w=2)[
                                :, :, hp, 1, jo:jo + 128]
                        q8 = q8T[gi][p0:p0 + 32, :].rearrange(
                            "p (s h w n) -> p s h w n", s=2, h=2, w=2)[
                            :, :, hp, 0, c0:TS]
                        nc.tensor.matmul(
                            ss[:, half * TS + c0:(half + 1) * TS],
                            k8, q8,
                            start=True, stop=True, perf_mode=DROW)
                u = upool.tile([128, 2 * TS], BF16, tag="u",
                               name=f"u_{gi}_{hp}_{jt}")
                utiles[gi, hp, jt] = u
                u3 = u[:].rearrange("p (h i) -> p h i", h=2)
                s3 = ss[:].rearrange("p (h i) -> p h i", h=2)
                if c0:
                    nc.scalar.activation(u3[:, :, c0:TS], s3[:, :, c0:TS], EXP)
                else:
                    nc.scalar.activation(u[:], ss[:], EXP)
                if d >= 0:
                    t3 = tri[:].rearrange("p (h m) -> p h m", h=2)
                    nc.vector.tensor_tensor(
                        u3[:, :, c0:c0 + 128], u3[:, :, c0:c0 + 128],
                        t3[:], op=MUL)
            return unit

        av_ps = {}

        def att_av(gi, ql, h):
            hp, half = h // 2, h % 2
            qtg = 4 * gi + ql

            def unit():
                if h == 0:
                    # cols 0-259: 4 heads x (64 attn-out | softmax sum);
                    # cols 264-391: the a->aT PE-transpose target (same bank)
                    av_ps[gi, ql] = ps_av.tile(
                        [128, 392], F32, tag="av", name=f"av_{gi}_{ql}")
                ps = av_ps[gi, ql]
                for jt in range(qtg + 1):
                    jts, jo = jt // 4, jt % 4
                    u3 = utiles[gi, hp, jt][:].rearrange(
                        "p (h i) -> p h i", h=2)
                    nc.tensor.matmul(
                        ps[:, h * 65:(h + 1) * 65],
                        u3[:, half, ql * 128:(ql + 1) * 128],
                        Vt[jts][:, jo * VW + h * 65:jo * VW + (h + 1) * 65],
                        start=(jt == 0), stop=(jt == qtg))
            return unit

        atiles = {}

        def att_norm(gi, ql):
            def unit():
                ps = av_ps[gi, ql]
                ps3 = ps[:, 0:260].rearrange("p (h e) -> p h e", e=DH + 1)
                r = rpool.tile([128, 4], F32, tag="r", name=f"r_{gi}_{ql}")
                nc.vector.reciprocal(r[:], ps3[:, :, DH])
                for hp in range(2):
                    a = apool.tile([128, 128], F32, tag="a",
                                   name=f"a_{gi}_{ql}_{hp}")
                    atiles[gi, ql, hp] = a
                    for half in range(2):
                        h = 2 * hp + half
                        nc.vector.tensor_scalar(
                            a[:, half * DH:(half + 1) * DH],
                            ps3[:, h, 0:DH], r[:, h:h + 1], None, op0=MUL)
            return unit

        aTtiles = {}

        def att_tr(gi, ql, hp):
            tt = 4 * gi + ql

            def unit():
                # hp=0 -> the tail region; hp=1 reuses the av columns (norm,
                # emitted earlier, is their last reader)
                tp = av_ps[gi, ql][:, 264:392] if hp == 0 \
                    else av_ps[gi, ql][:, 0:128]
                nc.tensor.transpose(tp, atiles[gi, ql, hp][:], ident)
                aT = atp.tile([128, 128], F32R, tag="aT",
                              name=f"aT_{tt}_{hp}")
                aTtiles[tt, hp] = aT
                nc.vector.tensor_copy(aT[:], tp)
            return unit

        out_t = {}

        def proj_unit(tt, nt):
            def unit():
                ps = ps_mm.tile([128, TS], F32, tag="mm", name=f"pj_{tt}_{nt}")
                for c in range(2):
                    nc.tensor.matmul(
                        ps[:], aTtiles[tt, c][:],
                        wpt[:, c * D + nt * TS:c * D + (nt + 1) * TS],
                        start=(c == 0), stop=(c == 1))
                if nt == 0:
                    out_t[tt] = outp.tile([128, 2 * TS], BF16, tag="o",
                                          name=f"o_{tt}")
                o = out_t[tt]
                if tt >= 12:
                    # ACT is free once the exp stream drains; keep the tail
                    # copies off DVE, which is busy with norms there
                    nc.scalar.copy(o[:, nt * TS:(nt + 1) * TS], ps[:])
                else:
                    nc.vector.tensor_copy(o[:, nt * TS:(nt + 1) * TS], ps[:])
                if tt >= 6:
                    # tail tiles: stream each half as soon as it's copied
                    nc.sync.dma_start(
                        out=out_dram[tt][:, nt * TS:(nt + 1) * TS],
                        in_=o[:, nt * TS:(nt + 1) * TS])
                elif nt == 1:
                    nc.sync.dma_start(out=out_dram[tt], in_=o[:])
            return unit

        def qk_part(ts, ct, ks, ke, ps_box):
            def unit():
                if ks == 0:
                    ps_box[0] = ps_mm.tile([128, TS], F32, tag="mm",
                                           name=f"qk_{ts}_{ct}")
                ps = ps_box[0]
                if True:
                    # fp8 DoubleRow: one matmul covers a 256-channel fold
                    # at 0.5 cycles/row.  Slices 1-3's q/k feed the fp8
                    # score path anyway; slice 0 keeps bf16 scores, paying
                    # only the ~3.5% input quantization on its projections.
                    j = ks // 2
                    w8v = wqk8[:, j * 2 * CQK:(j + 1) * 2 * CQK].rearrange(
                        "p (s m) -> p s m", s=2)
                    x8v = xr8[ts][:, j * 2 * TS:(j + 1) * 2 * TS].rearrange(
                        "p (s n) -> p s n", s=2)
                    nc.tensor.matmul(
                        ps[:], w8v[:, :, ct * 128:(ct + 1) * 128], x8v,
                        start=(ks == 0), stop=(ke == KT), perf_mode=DROW)
                if ke == KT:
                    emit_qk_copies(ts, ct, ps)
            return unit

        def qk_units(ts, prio_base=None, prio_step=None):
            # The qk path gates the next slice's exp stream, so it outranks
            # av/norm work.  k-chunks of 2 let accumulation start as soon
            # as the first xr tiles land instead of waiting for all 8;
            # <=2 psum groups live at a time.  (0,2) first: finishing
            # q-half-0 AND k-half-0 lets the hp=0 score stream start
            # earliest.
            us = []
            if ts > 1:  # slices 0/1 xr are prefetched in the setup block
                us.append((load_xr(ts), "pha"))
            boxes = [[None] for _ in range(4)]
            for cts in ((0, 2), (1, 3)):
                for ks in range(0, KT, 2):
                    for ct in cts:
                        us.append((qk_part(ts, ct, ks, ks + 2, boxes[ct]),
                                   "pha"))
            if prio_base is not None:
                us = [(u, prio_base + j * prio_step)
                      for j, (u, _) in enumerate(us)]
            return us

        def v_units(ts):
            # v emission trails the NEXT slice's qk units so the shared
            # ps_mm rotation never couples a critical qk allocation behind
            # a v copy that is itself waiting on the cold wv DMA
            return [(v_group(ts, sub), "mid") for sub in range(4)]

        def attention_units(gi, proj_here):
            njt = 4 * (gi + 1)
            seq = [(jt, hp) for jt in range(njt) for hp in range(2)]
            depth = 4  # sc-units of slack between an exp and its av consumers
            pending = {}
            for ql in range(4):
                idx = 2 * (4 * gi + ql) + 1 + depth
                post = [(att_av(gi, ql, h), "mid") for h in range(4)]
                post.append((att_norm(gi, ql), "mid"))
                for hp in range(2):
                    post.append((att_tr(gi, ql, hp), "mid"))
                if proj_here:
                    tt = 4 * gi + ql
                    post += [(proj_unit(tt, nt), "mid") for nt in range(2)]
                pending.setdefault(min(idx, len(seq)), []).extend(post)
            us = []
            for i, (jt, hp) in enumerate(seq):
                us.append((att_sc(gi, hp, jt), "sc"))
                for p in pending.pop(i + 1, []):
                    us.append(p)
            for idx in sorted(pending):
                us.extend(pending[idx])
            return us

        # Scheduler priority bands (lower number = preferred among READY
        # instructions).  The exp stream is the scarce serial resource: sc
        # units get top priority so ACT never starves while a score tile is
        # producible; qkv-projection/av/norm work runs at middle priority;
        # c_proj + output copies are pure filler that should soak up PE idle
        # only when nothing pressing is ready.
        # pha > av/norm: a late qkv projection stalls the NEXT slice's whole
        # exp stream, while av work only gates pool rotations 30+ slots out
        BAND = {"sc": 1_000_000, "pha": 1_500_000,
                "mid": 2_000_000, "low": 3_000_000}
        counters = {"sc": 0, "pha": 0, "mid": 0, "low": 0}

        def emit(unit, band):
            # band may be an explicit integer priority: the scheduler picks
            # the lowest-priority READY instruction per engine, so
            # interleaving qk priorities between consecutive sc priorities
            # forces 1:1 placement inside the score stream
            if isinstance(band, int):
                tc.cur_priority = band
                unit()
                return
            tc.cur_priority = BAND[band] + counters[band]
            counters[band] += 100
            unit()

        def sc_band(u):
            return (u, "sc")

        # qk+v for slice 0 run alone (nothing to overlap with yet).
        # CORRECTNESS INVARIANT: v_units(ts) must be fully emitted before
        # stream ts starts — the tile framework orders readers after
        # writers by emission, and the av units of stream ts read Vt[ts].
        SCB = BAND["sc"]
        # alloc order qk(0), qk(1), v(0): the ps_mm slot rotation must never
        # chain the slice-1 q-projection behind a v copy
        for u, band in (qk_units(0, SCB - 600, 10)
                        + qk_units(1, SCB - 300, 10)
                        + v_units(0)):
            emit(u, band)
        # qk(gi) lands two windows early: during window w the PE is
        # ACT-paced with plenty of idle, so the w+2 projections complete
        # long before their exp stream must start — no boundary stall
        fillers = [
            v_units(1) + qk_units(2, SCB + 1510, 5),
            qk_units(3, SCB + 3110, 5) + v_units(2)
            + [(proj_unit(tt, nt), "mid") for tt in (0, 1) for nt in range(2)],
            v_units(3)
            + [(proj_unit(tt, nt), "mid")
               for tt in (2, 3, 4, 5, 6) for nt in range(2)],
            [(proj_unit(tt, nt), "mid")
             for tt in (7, 8, 9, 10, 11) for nt in range(2)],
        ]
        for gi in range(NTS):
            for u, band in interleave(attention_units(gi, gi == NTS - 1),
                                      fillers[gi]):
                emit(u, band)

    nc.finalize()
    return nc


_NC = None


def _get_nc():
    global _NC
    if _NC is None:
        _NC = build()
    return _NC


def _make_in_maps(x, W_attn, b_attn, W_proj):
    jj = np.arange(128, dtype=np.int64)[:, None]
    ii = np.arange(128, dtype=np.int64)[None, :]
    tri = (jj <= ii).astype(np.float32)
    ident = np.eye(128, dtype=np.float32)
    aux = np.ascontiguousarray(np.concatenate([tri, tri, ident], axis=1))

    shards = []
    for g in range(4):
        q_cols = W_attn[:, g * CV:(g + 1) * CV]
        k_cols = W_attn[:, D + g * CV:D + (g + 1) * CV]
        wqk = np.ascontiguousarray(
            np.concatenate([q_cols, k_cols], axis=1)).reshape(
            KT, 128, CQK).astype(ml_dtypes.bfloat16)
        wv = np.ascontiguousarray(
            W_attn[:, 2 * D + g * CV:2 * D + (g + 1) * CV]
            .reshape(KT, 128, CV).transpose(1, 0, 2)
            .reshape(128, KT * CV)).astype(ml_dtypes.bfloat16)
        wp = np.ascontiguousarray(
            W_proj[g * CV:(g + 1) * CV, :].reshape(2, 128, D)
            .transpose(1, 0, 2).reshape(128, 2 * D))
        wqkf = np.concatenate([q_cols, k_cols], axis=1)  # [1024, 512] f32
        wqk8 = np.ascontiguousarray(
            (wqkf * 32.0).reshape(KT // 2, 2, 128, CQK)
            .transpose(0, 2, 1, 3).reshape(KT // 2, 128, 2 * CQK)
        ).astype(ml_dtypes.float8_e4m3fn)
        bq = b_attn[g * CV:(g + 1) * CV] / 8.0
        bk = b_attn[D + g * CV:D + (g + 1) * CV]
        bqk = np.ascontiguousarray(
            np.concatenate([bq, bk]).reshape(4, 128).T).astype(np.float32)
        shards.append((wqk, wv, wp, bqk, wqk8))

    in_maps = []
    for b in range(B):
        xT = np.ascontiguousarray(x[b].T).reshape(KT, 128, NTS, TS)
        # [NTS, 128, KT*TS]
        xT = np.ascontiguousarray(
            xT.transpose(2, 1, 0, 3).reshape(NTS, 128, KT * TS)
        ).astype(ml_dtypes.bfloat16)
        xb = np.ascontiguousarray(x[b].T)      # [1024 ch, 2048 tok]
        x8 = np.ascontiguousarray(
            xb.reshape(KT // 2, 2, 128, NTS, TS)
            .transpose(3, 2, 0, 1, 4).reshape(NTS, 128, KT * TS)
        ).astype(ml_dtypes.float8_e4m3fn)
        for g in range(4):
            wqk, wv, wp, bqk, wqk8 = shards[g]
            in_maps.append({
                "xT": xT, "wqk": wqk, "wv": wv, "wp": wp,
                "bqk": bqk, "aux": aux, "x8": x8, "wqk8": wqk8,
            })
    return in_maps


def run(inputs, trace=False):
    x = np.asarray(inputs["x"], dtype=np.float32)
    W_attn = np.asarray(inputs["W_attn"], dtype=np.float32)
    b_attn = np.asarray(inputs["b_attn"], dtype=np.float32)
    W_proj = np.asarray(inputs["W_proj"], dtype=np.float32)
    b_proj = np.asarray(inputs["b_proj"], dtype=np.float32)

    nc = _get_nc()
    in_maps = _make_in_maps(x, W_attn, b_attn, W_proj)
    res = run_bass_kernel_spmd(nc, in_maps, list(range(8)), trace=trace)

    out = np.zeros((B, T, D), dtype=np.float32)
    for b in range(B):
        for g in range(4):
            o = np.asarray(res.results[b * 4 + g]["out"])  # [16, 128, 1024]
            out[b] += o.astype(np.float32).reshape(T, D)
    # v-bias contributes a constant shift through the value path; b_proj too.
    const = b_attn[2 * D:3 * D] @ W_proj + b_proj
    out += const[None, None, :].astype(np.float32)
    return out, res


def kernel(**inputs):
    out, _ = run(inputs, trace=False)
    return out


# revision 141
# speedup vs baseline: 1.0011x; 1.0011x over previous
"""Causal multi-head attention block (GPT-style) on 8 TRN2 NeuronCores.

Sharding: core (b, g) = batch b in {0,1} x head-group g in {0..3} (4 heads of
dh=64 each). Megatron-style: each core computes q/k/v projections for its 256
channels, attention for its 4 heads, and a partial c_proj using its 256 rows of
W_proj. Host sums the 4 partial projections per batch (+ bias terms).

On-core dataflow.  Precision plan: all q/k projections run fp8e4m3
DoubleRow (host-prefolded 256-channel pairs, W pre-scaled x32 out of the
subnormal range, 0.5 PE cycles/row); slices 1-3 also SCORE in fp8
DoubleRow — their >=512-key softmaxes average the ~3% noise away — while
slice 0's near-diagonal scores stay bf16.  The V path and x's v-copy stay
bf16 end to end (v errors hit the output absmax directly); output is bf16.
  qT,kT = (W8 stationary) @ x8          -> psum -> bf16 (slice 0) and fp8
                                           key-pair tiles [32p, 2, .] built
                                           with 8 small SBUF->SBUF DMAs
  v     = (xT stationary) @ W_v         -> [2048, 4*65] bf16 (ones col appended)
  sT    = kT_tile.T @ qT_slice          -> scores transposed [j, i],
                                           exact causal column range
  u     = exp(sT) -> bf16 (no max-subtraction: scores are O(3); diagonal
                    boundary masked by a [128,128] triangle multiply)
  av    = (u_tile stationary [k, q128]) @ (v|1 moving [k, 65])
          -> psum [128 q, 4 heads x 65]: cols 0-63 unnormalized attn-out,
          col 64 = softmax sum.  Query-stationary orientation: 65-col moving
          operands cost 65 PE cycles each vs 512 for the [65, q] orientation.
  a     = av[:, h*65:h*65+64] * (1/av[:, h*65+64]) per-partition scalar (DVE)
  aT    = PE transpose(a)               -> [128 d, 128 q] per (qt, head-pair)
  out   = (aT stationary) @ W_proj_rows -> partial [2048, 1024] -> bf16 out

Scheduling: the ACT exp stream is the scarce serial resource.  Scheduler
priorities (lower wins among READY instructions) keep it fed: each slice's
q/k projection chain (matmuls, psum->sbuf copies, fp8 rearrange DMAs) is
placed ahead of or inside the score band so a slice boundary never starves
ACT; v/av/norm run at middle priority; all c_proj work floats at the back
as PE filler for the ACT-heavy late windows.  Engine placement respects
hardware limits (GPSIMD cannot touch PSUM): psum reads go to DVE/ACT, with
the exp-idle windows (early slices, post-exp tail) absorbing copies on ACT.
DMAs are packed into few fat transfers (dispatch is ~650ns serialized per
dma_start regardless of size), ordered x(slice0)+Wqk -> x(slice1) -> Wv ->
aux -> Wp by first-use time.
"""

import sys

try:
    import concourse  # noqa: F401
except ImportError:
    sys.path.insert(0, "/opt/trn_rl_repo")

from contextlib import ExitStack

import numpy as np

import concourse.tile as tile
from concourse import bacc, mybir
from concourse.bass_utils import run_bass_kernel_spmd

import ml_dtypes

F32 = mybir.dt.float32
F32R = mybir.dt.float32r
BF16 = mybir.dt.bfloat16
FP8 = mybir.dt.float8e4
EXP = mybir.ActivationFunctionType.Exp
MUL = mybir.AluOpType.mult
ADD = mybir.AluOpType.add
DROW = mybir.MatmulPerfMode.DoubleRow

B, T, D = 2, 2048, 1024
HG, DH = 4, 64          # heads per core, head dim
CQK = 512               # q+k channels per core
CV = 256                # v channels per core
KT = D // 128           # contraction tiles of the projections
TS = 512                # t-slice width
NTS = T // TS
NT128 = T // 128
VW = HG * (DH + 1)      # 260: per-t-tile v row (4 heads x (64 v + 1 ones))


def interleave(primary, filler, back=0.45):
    """Merge filler among primary units, weighted toward the back where the
    ACT pipeline debt is largest."""
    if not filler:
        return list(primary)
    n = len(primary)
    out = []
    fi = 0
    for i, p in enumerate(primary):
        out.append(p)
        x = (i + 1) / n
        want = ((1 - back) * x + back * x * x) * len(filler)
        while fi < len(filler) and fi + 1 <= want:
            out.append(filler[fi])
            fi += 1
    out.extend(filler[fi:])
    return out


def build():
    nc = bacc.Bacc(None)

    # DMA dispatch is ~650ns of serialized sequencer time per dma_start
    # regardless of size, so inputs are packed for few, fat transfers
    xT_in = nc.dram_tensor("xT", [NTS, 128, KT * TS], BF16, kind="ExternalInput")
    wqk_in = nc.dram_tensor("wqk", [KT, 128, CQK], BF16, kind="ExternalInput")
    wv_in = nc.dram_tensor("wv", [128, KT * CV], BF16, kind="ExternalInput")
    wp_in = nc.dram_tensor("wp", [128, 2 * D], F32R, kind="ExternalInput")
    x8_in = nc.dram_tensor("x8", [NTS, 128, KT * TS], FP8,
                           kind="ExternalInput")
    wqk8_in = nc.dram_tensor("wqk8", [KT // 2, 128, 2 * CQK], FP8,
                             kind="ExternalInput")
    bias_in = nc.dram_tensor("bqk", [128, 4], F32, kind="ExternalInput")
    aux_in = nc.dram_tensor("aux", [128, 384], F32, kind="ExternalInput")
    out_dram = nc.dram_tensor("out", [NT128, 128, 2 * TS], BF16,
                              kind="ExternalOutput")

    with ExitStack() as ctx:
        tc = ctx.enter_context(tile.TileContext(nc))

        const = ctx.enter_context(tc.tile_pool(name="const", bufs=1))
        big = ctx.enter_context(tc.tile_pool(name="big", bufs=1))
        upool = ctx.enter_context(tc.tile_pool(name="upool", bufs=44))
        apool = ctx.enter_context(tc.tile_pool(name="apool", bufs=6))
        atp = ctx.enter_context(tc.tile_pool(name="atp", bufs=33))
        rpool = ctx.enter_context(tc.tile_pool(name="rpool", bufs=4))
        outp = ctx.enter_context(tc.tile_pool(name="outp", bufs=8))
        xrp = ctx.enter_context(tc.tile_pool(name="xrp", bufs=2))
        wqkp = ctx.enter_context(tc.tile_pool(name="wqkp", bufs=1))
        x8p = ctx.enter_context(tc.tile_pool(name="x8p", bufs=2))
        wvp = ctx.enter_context(tc.tile_pool(name="wvp", bufs=1))

        ps_s = ctx.enter_context(tc.tile_pool(name="ps_s", bufs=2, space="PSUM"))
        ps_av = ctx.enter_context(tc.tile_pool(name="ps_av", bufs=2, space="PSUM"))
        ps_mm = ctx.enter_context(tc.tile_pool(name="ps_mm", bufs=2, space="PSUM"))

        # constants
        bias_sb = const.tile([128, 4], F32, tag="bias")
        auxf = const.tile([128, 384], F32, tag="auxf")  # [tri|tri|ident]
        tri = const.tile([128, 256], BF16, tag="tri")
        ident = auxf[:, 256:384]
        ones128 = const.tile([128, 16], BF16, tag="ones128")
        nc.vector.memset(ones128[:], 1.0)
        # preload the ACT exp table off the critical path
        scratch = const.tile([128, 16], BF16, tag="scratch")
        nc.scalar.activation(scratch[:], ones128[:], EXP)

        # persistent intermediates.  bf16 q/k only for slice 0 (its
        # near-diagonal softmaxes average over few keys, so quantization
        # error shows); slices 1-3 score in fp8 DoubleRow, where the ~3%
        # weight error washes out over >=512-key softmaxes.
        qkT = {(ct, 0): big.tile([128, TS], BF16, tag=f"qkT{ct}_0",
                                 name=f"qkT{ct}_0")
               for ct in range(4)}
        # fp8 key-pair layout, one tile per slice: partition half*32+p
        # holds channels half*64+p (s=0 half of the row) and
        # half*64+32+p (s=1); within a half the ct blocks are side by side
        # (slice 0 stores only its k blocks: cts 2,3)
        q8T = {}
        qk8s = ctx.enter_context(tc.tile_pool(name="qk8s", bufs=3))
        q8stage = {}
        Vt = [big.tile([128, 4 * VW], BF16, tag=f"Vt{ts}", name=f"Vt{ts}")
              for ts in range(NTS)]

        # weight + first-slice DMAs.  Slice 0 streams in 8 k-tile chunks so
        # accumulation can start as soon as the first pairs land; everything
        # else is one fat transfer per tensor (dispatch is the scarce
        # resource, not bandwidth).
        xrt, xr8 = {}, {}
        wqk8 = wqkp.tile([128, KT * CQK], FP8, tag="wqk8", name="wqk8")
        xr8[0] = x8p.tile([128, KT * TS], FP8, tag="x8", name="x8_0")
        for j in range(KT // 2):
            nc.sync.dma_start(
                out=xr8[0][:, j * 2 * TS:(j + 1) * 2 * TS],
                in_=x8_in[0][:, j * 2 * TS:(j + 1) * 2 * TS])
            nc.sync.dma_start(
                out=wqk8[:, j * 2 * CQK:(j + 1) * 2 * CQK],
                in_=wqk8_in[j])
        xrt[0] = xrp.tile([128, KT * TS], BF16, tag="xr", name="xr_0")
        # v-path + cold weights ride the SECOND hardware DGE queue (ACT),
        # held until the critical fp8 qk stream has cleared the DMA engine;
        # their dispatches finish on ACT.SEQ before the first exp fires
        with tc.tile_wait_until(0.001):
            for k in range(0, KT, 4):
                nc.scalar.dma_start(out=xrt[0][:, k * TS:(k + 4) * TS],
                                    in_=xT_in[0][:, k * TS:(k + 4) * TS])
        # xr slice 1 right behind slice 0: the slice-1 q-projection gates
        # that slice's whole exp stream, while wv/aux/wp aren't read
        # until later
        nc.sync.dma_start(out=bias_sb[:], in_=bias_in[:])
        xr8[1] = x8p.tile([128, KT * TS], FP8, tag="x8", name="x8_1")
        nc.sync.dma_start(out=xr8[1][:], in_=x8_in[1])
        nc.sync.dma_start(out=auxf[:], in_=aux_in[:])
        xrt[1] = xrp.tile([128, KT * TS], BF16, tag="xr", name="xr_1")
        wvt = wvp.tile([128, KT * CV], BF16, tag="wv", name="wvt")
        wpt = big.tile([128, 2 * D], F32R, tag="wpt", name="wpt")
        with tc.tile_wait_until(0.001):
            for k in range(0, KT, 4):
                nc.scalar.dma_start(out=xrt[1][:, k * TS:(k + 4) * TS],
                                    in_=xT_in[1][:, k * TS:(k + 4) * TS])
            nc.scalar.dma_start(out=wvt[:], in_=wv_in[:])
            nc.scalar.dma_start(out=wpt[:], in_=wp_in[:])
        nc.gpsimd.tensor_copy(tri[:], auxf[:, 0:256])

        def load_xr(ts):
            def unit():
                x1 = xrp.tile([128, KT * TS], BF16, tag="xr", name=f"xr_{ts}")
                for k in range(0, KT, 4):
                    nc.sync.dma_start(out=x1[:, k * TS:(k + 4) * TS],
                                      in_=xT_in[ts][:, k * TS:(k + 4) * TS])
                xrt[ts] = x1
                x2 = x8p.tile([128, KT * TS], FP8, tag="x8", name=f"x8_{ts}")
                nc.sync.dma_start(out=x2[:], in_=x8_in[ts])
                xr8[ts] = x2
            return unit

        q8n = {}

        def emit_qk_copies(ts, ct, ps):
            # The psum->sbuf copies and rearrange DMAs sit on the critical
            # chain to the next slice's exp stream AND gate the ps_mm slot
            # rotation; never let them lose the DVE/queue priority heaps to
            # bulk sc-band work.
            saved_prio = tc.cur_priority
            tc.cur_priority = BAND["sc"] - 10_000 + 10 * (4 * ts + ct)
            """psum -> bf16 (slice 0 only) and/or the fp8 stage tile.  The
            key-pair rearrange DMAs fire per head-pair as soon as that
            pair's q and k blocks are staged (ct order is 0,2,1,3), so the
            hp=0 score stream never waits on the hp=1 projections."""
            scale = 0.125 if ct < 2 else 1.0
            scale /= 32.0   # wqk8 is pre-scaled x32 into fp8 normal range
            if ts == 0:
                nc.vector.tensor_scalar(
                    qkT[ct, 0][:], ps[:],
                    scale, bias_sb[:, ct:ct + 1], op0=MUL, op1=ADD)
            if ct >= 2 or ts >= 1:
                cdim = 2 if ts == 0 else 4
                cx = ct - 2 if ts == 0 else ct
                if ts not in q8stage:
                    q8stage[ts] = qk8s.tile([128, cdim * TS], FP8,
                                            tag="qk8s", name=f"qk8s_{ts}")
                    q8T[ts] = big.tile([128, 2 * cdim * TS], FP8,
                                       tag=f"q8T{ts}", name=f"q8T{ts}")
                    q8n[ts] = 0
                st = q8stage[ts]
                nc.vector.tensor_scalar(
                    st[:, cx * TS:(cx + 1) * TS], ps[:],
                    scale, bias_sb[:, ct:ct + 1], op0=MUL, op1=ADD)
                q8n[ts] += 1
                if ts == 0:
                    # k-only tile [64, (s, h, n)]; after each ct the matching
                    # hp's 4 DMAs can go
                    hp = ct - 2
                elif q8n[ts] == 2:
                    hp = 0   # ct order is 0,2,1,3: q+k of pair 0 staged
                elif q8n[ts] == 4:
                    hp = 1
                else:
                    tc.cur_priority = saved_prio
                    return
                if ts == 0:
                    o4 = q8T[0][:].rearrange("p (s h n) -> p s h n",
                                             s=2, h=2)
                    for half in range(2):
                        for s in range(2):
                            nc.sync.dma_start(
                                out=o4[half * 32:(half + 1) * 32, s, hp],
                                in_=st[half * 64 + s * 32:
                                       half * 64 + s * 32 + 32,
                                       hp * TS:(hp + 1) * TS])
                else:
                    # [64, (s, h, w, n)]: w = q|k for that head pair
                    o5 = q8T[ts][:].rearrange("p (s h w n) -> p s h w n",
                                              s=2, h=2, w=2)
                    i4 = st[:].rearrange("p (c n) -> p c n", c=4)
                    for half in range(2):
                        for s in range(2):
                            nc.sync.dma_start(
                                out=o5[half * 32:(half + 1) * 32, s, hp],
                                in_=i4[half * 64 + s * 32:
                                       half * 64 + s * 32 + 32, hp::2])
            tc.cur_priority = saved_prio

        def v_group(ts, sub):
            def unit():
                ps = ps_mm.tile([128, CV], F32, tag="mm", name=f"v_{ts}_{sub}")
                for k in range(KT):
                    nc.tensor.matmul(
                        ps[:],
                        xrt[ts][:, k * TS + sub * 128:k * TS + sub * 128 + 128],
                        wvt[:, k * CV:(k + 1) * CV],
                        start=(k == 0), stop=(k == KT - 1))
                v3 = Vt[ts][:].rearrange("p (s h e) -> p s h e", h=HG, e=DH + 1)
                nc.vector.tensor_copy(
                    v3[:, sub, :, 0:DH],
                    ps[:].rearrange("p (h e) -> p h e", e=DH))
                if sub == 0:
                    nc.vector.tensor_copy(
                        v3[:, :, :, DH],
                        ones128[:].rearrange("p (s h) -> p s h", h=HG))
            return unit

        utiles = {}

        def att_sc(gi, hp, jt):
            def unit():
                d = jt * 128 - gi * TS
                c0 = max(d, 0)
                ss = ps_s.tile([128, 2 * TS], F32, tag="ss",
                               name=f"ss_{gi}_{hp}_{jt}")
                jts = jt // 4
                jo = (jt % 4) * 128
                for half in range(2):
                    if gi == 0:
                        p0 = half * 64
                        nc.tensor.matmul(
                            ss[:, half * TS + c0:(half + 1) * TS],
                            qkT[2 + hp, jts][p0:p0 + 64, jo:jo + 128],
                            qkT[hp, gi][p0:p0 + 64, c0:TS],
                            start=True, stop=True)
                    else:
                        p0 = half * 32
                        if jts == 0:
                            k8 = q8T[0][p0:p0 + 32, :].rearrange(
                                "p (s h n) -> p s h n", s=2, h=2)[
                                :, :, hp, jo:jo + 128]
                        else:
                            k8 = q8T[jts][p0:p0 + 32, :].rearrange(
                                "p (s h w n) -> p s h w n", s=2, h=2, w=2)[
                                :, :, hp, 1, jo:jo + 128]
                        q8 = q8T[gi][p0:p0 + 32, :].rearrange(
                            "p (s h w n) -> p s h w n", s=2, h=2, w=2)[
                            :, :, hp, 0, c0:TS]
                        nc.tensor.matmul(
                            ss[:, half * TS + c0:(half + 1) * TS],
                            k8, q8,
                            start=True, stop=True, perf_mode=DROW)
                u = upool.tile([128, 2 * TS], BF16, tag="u",
                               name=f"u_{gi}_{hp}_{jt}")
                utiles[gi, hp, jt] = u
                u3 = u[:].rearrange("p (h i) -> p h i", h=2)
                s3 = ss[:].rearrange("p (h i) -> p h i", h=2)
                if c0:
                    nc.scalar.activation(u3[:, :, c0:TS], s3[:, :, c0:TS], EXP)
                else:
                    nc.scalar.activation(u[:], ss[:], EXP)
                if d >= 0:
                    t3 = tri[:].rearrange("p (h m) -> p h m", h=2)
                    nc.vector.tensor_tensor(
                        u3[:, :, c0:c0 + 128], u3[:, :, c0:c0 + 128],
                        t3[:], op=MUL)
            return unit

        av_ps = {}

        def att_av(gi, ql, h):
            hp, half = h // 2, h % 2
            qtg = 4 * gi + ql

            def unit():
                if h == 0:
                    # cols 0-259: 4 heads x (64 attn-out | softmax sum);
                    # cols 264-391: the a->aT PE-transpose target (same bank)
                    av_ps[gi, ql] = ps_av.tile(
                        [128, 392], F32, tag="av", name=f"av_{gi}_{ql}")
                ps = av_ps[gi, ql]
                for jt in range(qtg + 1):
                    jts, jo = jt // 4, jt % 4
                    u3 = utiles[gi, hp, jt][:].rearrange(
                        "p (h i) -> p h i", h=2)
                    nc.tensor.matmul(
                        ps[:, h * 65:(h + 1) * 65],
                        u3[:, half, ql * 128:(ql + 1) * 128],
                        Vt[jts][:, jo * VW + h * 65:jo * VW + (h + 1) * 65],
                        start=(jt == 0), stop=(jt == qtg))
            return unit

        atiles = {}

        def att_norm(gi, ql):
            def unit():
                ps = av_ps[gi, ql]
                ps3 = ps[:, 0:260].rearrange("p (h e) -> p h e", e=DH + 1)
                r = rpool.tile([128, 4], F32, tag="r", name=f"r_{gi}_{ql}")
                nc.vector.reciprocal(r[:], ps3[:, :, DH])
                for hp in range(2):
                    a = apool.tile([128, 128], F32, tag="a",
                                   name=f"a_{gi}_{ql}_{hp}")
                    atiles[gi, ql, hp] = a
                    for half in range(2):
                        h = 2 * hp + half
                        nc.vector.tensor_scalar(
                            a[:, half * DH:(half + 1) * DH],
                            ps3[:, h, 0:DH], r[:, h:h + 1], None, op0=MUL)
            return unit

        aTtiles = {}

        def att_tr(gi, ql, hp):
            tt = 4 * gi + ql

            def unit():
                # hp=0 -> the tail region; hp=1 reuses the av columns (norm,
                # emitted earlier, is their last reader)
                tp = av_ps[gi, ql][:, 264:392] if hp == 0 \
                    else av_ps[gi, ql][:, 0:128]
                nc.tensor.transpose(tp, atiles[gi, ql, hp][:], ident)
                aT = atp.tile([128, 128], F32R, tag="aT",
                              name=f"aT_{tt}_{hp}")
                aTtiles[tt, hp] = aT
                nc.vector.tensor_copy(aT[:], tp)
            return unit

        out_t = {}

        def proj_unit(tt, nt):
            def unit():
                ps = ps_mm.tile([128, TS], F32, tag="mm", name=f"pj_{tt}_{nt}")
                for c in range(2):
                    nc.tensor.matmul(
                        ps[:], aTtiles[tt, c][:],
                        wpt[:, c * D + nt * TS:c * D + (nt + 1) * TS],
                        start=(c == 0), stop=(c == 1))
                if nt == 0:
                    out_t[tt] = outp.tile([128, 2 * TS], BF16, tag="o",
                                          name=f"o_{tt}")
                o = out_t[tt]
                if tt >= 12:
                    # ACT is free once the exp stream drains; keep the tail
                    # copies off DVE, which is busy with norms there
                    nc.scalar.copy(o[:, nt * TS:(nt + 1) * TS], ps[:])
                else:
                    nc.vector.tensor_copy(o[:, nt * TS:(nt + 1) * TS], ps[:])
                if tt >= 6:
                    # tail tiles: stream each half as soon as it's copied
                    nc.sync.dma_start(
                        out=out_dram[tt][:, nt * TS:(nt + 1) * TS],
                        in_=o[:, nt * TS:(nt + 1) * TS])
                elif nt == 1:
                    nc.sync.dma_start(out=out_dram[tt], in_=o[:])
            return unit

        def qk_part(ts, ct, ks, ke, ps_box):
            def unit():
                if ks == 0:
                    ps_box[0] = ps_mm.tile([128, TS], F32, tag="mm",
                                           name=f"qk_{ts}_{ct}")
                ps = ps_box[0]
                if True:
                    # fp8 DoubleRow: one matmul covers a 256-channel fold
                    # at 0.5 cycles/row.  Slices 1-3's q/k feed the fp8
                    # score path anyway; slice 0 keeps bf16 scores, paying
                    # only the ~3.5% input quantization on its projections.
                    j = ks // 2
                    w8v = wqk8[:, j * 2 * CQK:(j + 1) * 2 * CQK].rearrange(
                        "p (s m) -> p s m", s=2)
                    x8v = xr8[ts][:, j * 2 * TS:(j + 1) * 2 * TS].rearrange(
                        "p (s n) -> p s n", s=2)
                    nc.tensor.matmul(
                        ps[:], w8v[:, :, ct * 128:(ct + 1) * 128], x8v,
                        start=(ks == 0), stop=(ke == KT), perf_mode=DROW)
                if ke == KT:
                    emit_qk_copies(ts, ct, ps)
            return unit

        def qk_units(ts, prio_base=None, prio_step=None):
            # The qk path gates the next slice's exp stream, so it outranks
            # av/norm work.  k-chunks of 2 let accumulation start as soon
            # as the first xr tiles land instead of waiting for all 8;
            # <=2 psum groups live at a time.  (0,2) first: finishing
            # q-half-0 AND k-half-0 lets the hp=0 score stream start
            # earliest.
            us = []
            if ts > 1:  # slices 0/1 xr are prefetched in the setup block
                us.append((load_xr(ts), "pha"))
            boxes = [[None] for _ in range(4)]
            for cts in ((0, 2), (1, 3)):
                for ks in range(0, KT, 2):
                    for ct in cts:
                        us.append((qk_part(ts, ct, ks, ks + 2, boxes[ct]),
                                   "pha"))
            if prio_base is not None:
                us = [(u, prio_base + j * prio_step)
                      for j, (u, _) in enumerate(us)]
            return us

        def v_units(ts):
            # v emission trails the NEXT slice's qk units so the shared
            # ps_mm rotation never couples a critical qk allocation behind
            # a v copy that is itself waiting on the cold wv DMA
            return [(v_group(ts, sub), "mid") for sub in range(4)]

        def attention_units(gi, proj_here):
            njt = 4 * (gi + 1)
            seq = [(jt, hp) for jt in range(njt) for hp in range(2)]
            depth = 2  # sc-units of slack between an exp and its av consumers
            pending = {}
            for ql in range(4):
                idx = 2 * (4 * gi + ql) + 1 + depth
                post = [(att_av(gi, ql, h), "mid") for h in range(4)]
                post.append((att_norm(gi, ql), "mid"))
                for hp in range(2):
                    post.append((att_tr(gi, ql, hp), "mid"))
                if proj_here:
                    tt = 4 * gi + ql
                    post += [(proj_unit(tt, nt), "mid") for nt in range(2)]
                pending.setdefault(min(idx, len(seq)), []).extend(post)
            us = []
            for i, (jt, hp) in enumerate(seq):
                us.append((att_sc(gi, hp, jt), "sc"))
                for p in pending.pop(i + 1, []):
                    us.append(p)
            for idx in sorted(pending):
                us.extend(pending[idx])
            return us

        # Scheduler priority bands (lower number = preferred among READY
        # instructions).  The exp stream is the scarce serial resource: sc
        # units get top priority so ACT never starves while a score tile is
        # producible; qkv-projection/av/norm work runs at middle priority;
        # c_proj + output copies are pure filler that should soak up PE idle
        # only when nothing pressing is ready.
        # pha > av/norm: a late qkv projection stalls the NEXT slice's whole
        # exp stream, while av work only gates pool rotations 30+ slots out
        BAND = {"sc": 1_000_000, "pha": 1_500_000,
                "mid": 2_000_000, "low": 3_000_000}
        counters = {"sc": 0, "pha": 0, "mid": 0, "low": 0}

        def emit(unit, band):
            # band may be an explicit integer priority: the scheduler picks
            # the lowest-priority READY instruction per engine, so
            # interleaving qk priorities between consecutive sc priorities
            # forces 1:1 placement inside the score stream
            if isinstance(band, int):
                tc.cur_priority = band
                unit()
                return
            tc.cur_priority = BAND[band] + counters[band]
            counters[band] += 100
            unit()

        def sc_band(u):
            return (u, "sc")

        # qk+v for slice 0 run alone (nothing to overlap with yet).
        # CORRECTNESS INVARIANT: v_units(ts) must be fully emitted before
        # stream ts starts — the tile framework orders readers after
        # writers by emission, and the av units of stream ts read Vt[ts].
        SCB = BAND["sc"]
        # alloc order qk(0), qk(1), v(0): the ps_mm slot rotation must never
        # chain the slice-1 q-projection behind a v copy
        for u, band in (qk_units(0, SCB - 600, 10)
                        + qk_units(1, SCB - 300, 10)
                        + v_units(0)):
            emit(u, band)
        # qk(gi) lands two windows early: during window w the PE is
        # ACT-paced with plenty of idle, so the w+2 projections complete
        # long before their exp stream must start — no boundary stall
        fillers = [
            v_units(1) + qk_units(2, SCB + 1510, 5),
            qk_units(3, SCB + 3110, 5) + v_units(2)
            + [(proj_unit(tt, nt), "mid") for tt in (0, 1) for nt in range(2)],
            v_units(3)
            + [(proj_unit(tt, nt), "mid")
               for tt in (2, 3, 4, 5, 6) for nt in range(2)],
            [(proj_unit(tt, nt), "mid")
             for tt in (7, 8, 9, 10, 11) for nt in range(2)],
        ]
        for gi in range(NTS):
            for u, band in interleave(attention_units(gi, gi == NTS - 1),
                                      fillers[gi]):
                emit(u, band)

    nc.finalize()
    return nc


_NC = None


def _get_nc():
    global _NC
    if _NC is None:
        _NC = build()
    return _NC


def _make_in_maps(x, W_attn, b_attn, W_proj):
    jj = np.arange(128, dtype=np.int64)[:, None]
    ii = np.arange(128, dtype=np.int64)[None, :]
    tri = (jj <= ii).astype(np.float32)
    ident = np.eye(128, dtype=np.float32)
    aux = np.ascontiguousarray(np.concatenate([tri, tri, ident], axis=1))

    shards = []
    for g in range(4):
        q_cols = W_attn[:, g * CV:(g + 1) * CV]
        k_cols = W_attn[:, D + g * CV:D + (g + 1) * CV]
        wqk = np.ascontiguousarray(
            np.concatenate([q_cols, k_cols], axis=1)).reshape(
            KT, 128, CQK).astype(ml_dtypes.bfloat16)
        wv = np.ascontiguousarray(
            W_attn[:, 2 * D + g * CV:2 * D + (g + 1) * CV]
            .reshape(KT, 128, CV).transpose(1, 0, 2)
            .reshape(128, KT * CV)).astype(ml_dtypes.bfloat16)
        wp = np.ascontiguousarray(
            W_proj[g * CV:(g + 1) * CV, :].reshape(2, 128, D)
            .transpose(1, 0, 2).reshape(128, 2 * D))
        wqkf = np.concatenate([q_cols, k_cols], axis=1)  # [1024, 512] f32
        wqk8 = np.ascontiguousarray(
            (wqkf * 32.0).reshape(KT // 2, 2, 128, CQK)
            .transpose(0, 2, 1, 3).reshape(KT // 2, 128, 2 * CQK)
        ).astype(ml_dtypes.float8_e4m3fn)
        bq = b_attn[g * CV:(g + 1) * CV] / 8.0
        bk = b_attn[D + g * CV:D + (g + 1) * CV]
        bqk = np.ascontiguousarray(
            np.concatenate([bq, bk]).reshape(4, 128).T).astype(np.float32)
        shards.append((wqk, wv, wp, bqk, wqk8))

    in_maps = []
    for b in range(B):
        xT = np.ascontiguousarray(x[b].T).reshape(KT, 128, NTS, TS)
        # [NTS, 128, KT*TS]
        xT = np.ascontiguousarray(
            xT.transpose(2, 1, 0, 3).reshape(NTS, 128, KT * TS)
        ).astype(ml_dtypes.bfloat16)
        xb = np.ascontiguousarray(x[b].T)      # [1024 ch, 2048 tok]
        x8 = np.ascontiguousarray(
            xb.reshape(KT // 2, 2, 128, NTS, TS)
            .transpose(3, 2, 0, 1, 4).reshape(NTS, 128, KT * TS)
        ).astype(ml_dtypes.float8_e4m3fn)
        for g in range(4):
            wqk, wv, wp, bqk, wqk8 = shards[g]
            in_maps.append({
                "xT": xT, "wqk": wqk, "wv": wv, "wp": wp,
                "bqk": bqk, "aux": aux, "x8": x8, "wqk8": wqk8,
            })
    return in_maps


def run(inputs, trace=False):
    x = np.asarray(inputs["x"], dtype=np.float32)
    W_attn = np.asarray(inputs["W_attn"], dtype=np.float32)
    b_attn = np.asarray(inputs["b_attn"], dtype=np.float32)
    W_proj = np.asarray(inputs["W_proj"], dtype=np.float32)
    b_proj = np.asarray(inputs["b_proj"], dtype=np.float32)

    nc = _get_nc()
    in_maps = _make_in_maps(x, W_attn, b_attn, W_proj)
    res = run_bass_kernel_spmd(nc, in_maps, list(range(8)), trace=trace)

    out = np.zeros((B, T, D), dtype=np.float32)
    for b in range(B):
        for g in range(4):
            o = np.asarray(res.results[b * 4 + g]["out"])  # [16, 128, 1024]
            out[b] += o.astype(np.float32).reshape(T, D)
    # v-bias contributes a constant shift through the value path; b_proj too.
    const = b_attn[2 * D:3 * D] @ W_proj + b_proj
    out += const[None, None, :].astype(np.float32)
    return out, res


def kernel(**inputs):
    out, _ = run(inputs, trace=False)
    return out
